# revision 6
# baseline (speedup 1.0000x reference)
"""SPDNet kernel for Trainium2 (8 NeuronCores, data-parallel over batch).

Math: the reference's spd_rectify stages are identity maps (input SPD matrices
have all eigenvalues >= 1 >> EPS_RECT, and Stiefel compressions keep the
spectrum inside [lambda_min, lambda_max] subset of [1.37, 2.94]).  So the
network collapses to
    h_b   = W^T x_b W,         W = W1 @ W2 @ W3           (400x50, orthonormal)
    S_b   = logm(h_b)          (eigenvalues of h in [1.377, 2.937])
    out_b = <S_b, G_o> + bias  (G folds the sqrt(2)-scaled triu vectorization
                                and the final linear layer)
logm is evaluated eigendecomposition-free as a degree-8 polynomial in
s = h - m*I (near-minimax Chebyshev fit of log(m+s) on the padded spectrum
range [1.35, 2.96]; max fit error 1.2e-7), via Paterson-Stockmeyer with
v = s^3:  p(s) = (C2(s)*v + C1(s))*v + C0(s),  C_g quadratic in s.

x_b is SYMMETRIC, so only its lower block-triangle is DMA'd (100-row chunks
with column widths 128/200/300/400; the 128 pad keeps descriptor runs >=512B).
That cuts the dominant HBM traffic from 640KB to 411KB per matrix.  h is then
assembled without ever materializing W^T x:
    per 100x100 block (r>=c):  P_rc = X_rc^T W_r      (x stationary, fp16 W
                                                       moving, 50 rows each)
    accumulate Psum_c = sum_{r>c} P_rc in PSUM, evict bank to fp16, then
    h = sum_c [Psum_c^T W_c + W_c^T Psum_c] + sum_k P_kk^T W_k - m I
(all step-2 matmuls are fp16 x fp16, 1 cycle/row at any width).  The
polynomial runs with fp16 power/stationary tiles and fp16-moving combines.
Final contraction <S_b, G_o>: elementwise mul on GpSimd, X-reduce on DVE,
partition-sum via ones-vector matmul on the tensor engine.
"""

import numpy as np

N_CORES = 8
B_FULL = 256
BC = B_FULL // N_CORES      # 32 per core
N_IN = 400
N_OUT = 50

# log(m + s) polynomial on s in [lo-m, hi-m], from Chebyshev interpolation
# (degree 8, domain [1.35, 2.96]); coefficients are monomial-basis in s.
M_SHIFT = 2.1550000000000002
COEF = [
    0.7677907235557108, 0.4640362223750899, -0.10766484774906421,
    0.03332547763901113, -0.011599509906866342, 0.004203545486868787,
    -0.0016222327568142045, 0.0008559664117230024, -0.0003500826285455622,
]

# lower-triangle row-chunk DMA widths (>=128 cols keeps runs >= 512B)
W_R = [128, 200, 300, 400]

# batch chunks (start, size); small last chunk shortens the serial tail
CHUNKS = [(0, 6), (6, 8), (14, 8), (22, 6), (28, 4)]

# P-bank region offsets: Psum_c (c=0,1,2) then P_kk (k=0..3)
PSUM_OFF = {"acc0": 0, "acc1": 50, "acc2": 100,
            "d0": 150, "d1": 200, "d2": 250, "d3": 300}
PBANK_W = 350

CFG = {"xp": 2, "pmp": 4, "sp": 3, "tp": 2, "rp": 2,
       "pb": 2, "ph": 2, "pm": 3}

_CACHE = {}


def _apply_tile_patch():
    """This container's walrus rejects instructions carrying more than a
    couple of semaphore waits ("Too many sync wait commands") which the Tile
    tail drain always does.  Split the drain's waits across one sync-engine
    nop per logical processor instead."""
    if _CACHE.get("patched"):
        return
    import concourse.tile as ctile
    from bass_rust import VectorClock, ScopedClock, N_PROCS

    def _drain_and_barrier_split(self, tick_clock, wait_clock):
        gc = tick_clock.global_clock
        for p in range(N_PROCS):
            if gc[p] == 0:
                continue
            sub = [gc[q] if q == p else 0 for q in range(N_PROCS)]
            nop_inst = self.nc.sync.nop(nofuse=True, hint=f"drain_split_{p}")
            wait_clock.add_sem_waits(
                nop_inst.ins, ScopedClock({None: VectorClock(sub)})
            )
        self.nc.sync.drain()  # waits already emitted on the nops above
        self.nc.all_engine_barrier()
        assert self.sems is not None
        popped = self.nc._tile_sem_poison_stack.pop()
        assert popped is self._sem_poison
        self.nc.clear_and_free_semaphores(list(self.sems.allocated().values()))
        self.nc.all_engine_barrier()

    ctile.TileContext._drain_and_barrier = _drain_and_barrier_split
    _CACHE["patched"] = True


def _split_excess_waits(nc, limit=1):
    """This container's walrus rejects instructions with more than `limit`
    semaphore waits.  Move excess waits onto same-engine nops inserted
    immediately before the instruction (identical stall semantics)."""
    import concourse.mybir as mybir

    n_split = 0
    for fn in nc.m.functions:
        for blk in fn.blocks:
            new_insts = []
            for inst in blk.instructions:
                si = getattr(inst, "sync_info", None)
                waits = list(si.on_wait) if si is not None and si.on_wait else []
                if len(waits) > limit:
                    extra, keep = waits[:-limit], waits[-limit:]
                    for ci, cs in enumerate(range(0, len(extra), limit)):
                        chunk = extra[cs: cs + limit]
                        nop = mybir.InstNoOp(
                            name=f"{inst.name}-ws{ci}", ins=[], outs=[]
                        )
                        nop.engine = inst.engine
                        nop.sync_info = mybir.SyncInfo(on_wait=chunk, on_update=[])
                        new_insts.append(nop)
                        n_split += 1
                    si.on_wait = keep
                new_insts.append(inst)
            if n_split:
                blk.instructions[:] = new_insts
    return n_split


def _build_program():
    import concourse.bass as bass
    import concourse.mybir as mybir
    from concourse import tile

    F32 = mybir.dt.float32
    F32R = mybir.dt.float32r
    FP16 = mybir.dt.float16
    nc = bass.Bass()
    x_d = nc.declare_dram_parameter("x", [BC, N_IN, N_IN], F32R, isOutput=False)
    w_d = nc.declare_dram_parameter("w", [100, 200], FP16, isOutput=False)
    g_d = nc.declare_dram_parameter("g", [50, 350], FP16, isOutput=False)
    ci_d = nc.declare_dram_parameter("ci", [50, 450], F32R, isOutput=False)
    it_d = nc.declare_dram_parameter("it", [50, 400], FP16, isOutput=False)
    c32_d = nc.declare_dram_parameter("c32", [50, 1], F32, isOutput=False)
    o_d = nc.declare_dram_parameter("out", [7 * BC], F32, isOutput=True)

    with tile.TileContext(nc) as tc:
        with (
            tc.tile_pool(name="const", bufs=1) as constp,
            tc.tile_pool(name="xp", bufs=CFG["xp"]) as xp,
            tc.tile_pool(name="pmp", bufs=CFG["pmp"]) as pmp,
            tc.tile_pool(name="sp", bufs=CFG["sp"]) as sp_pool,
            tc.tile_pool(name="tp", bufs=CFG["tp"]) as tp,
            tc.tile_pool(name="rp", bufs=CFG["rp"]) as rp,
            tc.tile_pool(name="op", bufs=1) as op_pool,
            tc.tile_pool(name="pb", bufs=CFG["pb"], space="PSUM") as pb,
            tc.tile_pool(name="ph", bufs=CFG["ph"], space="PSUM") as ph,
            tc.tile_pool(name="pm", bufs=CFG["pm"], space="PSUM") as pm,
            tc.tile_pool(name="pr", bufs=1, space="PSUM") as pr,
        ):
            wh = constp.tile([100, 200], FP16, tag="wh")
            nc.sync.dma_start(out=wh[:], in_=w_d[:])
            ci_t = constp.tile([50, 450], F32R, tag="ci")
            nc.gpsimd.dma_start(out=ci_t[:], in_=ci_d[:])
            it8 = constp.tile([50, 400], FP16, tag="it8")
            nc.gpsimd.dma_start(out=it8[:], in_=it_d[:])
            gt = constp.tile([50, 350], FP16, tag="gt")
            nc.gpsimd.dma_start(out=gt[:], in_=g_d[:])
            on32 = constp.tile([50, 1], F32, tag="on32")
            nc.gpsimd.dma_start(out=on32[:], in_=c32_d[:])

            cI = lambda k: ci_t[:, 50 * k: 50 * k + 50]
            # blocks: 0:-m, 1:a7, 2:a8, 3:a6, 4:a4, 5:a5, 6:a3, 7:a1, 8:a2
            Wc = lambda r: wh[:, 50 * r: 50 * r + 50]

            out_ps = pr.tile([1, 7 * BC], F32, tag="ops")
            import concourse.mybir as _mb

            def do_group(b0, gb, out_off):
                W_ = 50 * gb
                # ---- lower-triangle x DMA: 4 row-chunks for the whole group
                xts = []
                for r in range(4):
                    w = W_R[r]
                    xt = xp.tile([100, gb, w], F32R, tag=f"x{r}")
                    nc.sync.dma_start(
                        out=xt[:],
                        in_=x_d[b0: b0 + gb, 100 * r: 100 * r + 100, 0:w]
                        .rearrange("b p j -> p b j"),
                    )
                    xts.append(xt)

                hps = ph.tile([50, W_], F32, tag="h")

                def step1(bi):
                    pb_t = pb.tile([100, PBANK_W], F32, tag="pbk")
                    first = True
                    for r in range(4):
                        for c in range(r + 1):
                            off = PSUM_OFF[f"d{r}"] if c == r else PSUM_OFF[f"acc{c}"]
                            nc.tensor.matmul(
                                pb_t[:, off: off + 50],
                                lhsT=xts[r][:, bi, 100 * c: 100 * c + 100],
                                rhs=Wc(r),
                                start=first, stop=(r == 3 and c == 3),
                            )
                            first = False
                    pmt = pmp.tile([100, PBANK_W], FP16, tag="pmt")
                    nc.scalar.copy(pmt[:], pb_t[:])
                    return pmt

                def step2(bi, pmt, first_h):
                    sl = hps[:, 50 * bi: 50 * bi + 50]
                    mm = 0
                    for c in range(3):
                        acc = pmt[:, PSUM_OFF[f"acc{c}"]: PSUM_OFF[f"acc{c}"] + 50]
                        nc.tensor.matmul(sl, lhsT=Wc(c), rhs=acc,
                                         start=(first_h and mm == 0), stop=False)
                        mm += 1
                        nc.tensor.matmul(sl, lhsT=acc, rhs=Wc(c),
                                         start=False, stop=False)
                        mm += 1
                    for k in range(4):
                        dk = pmt[:, PSUM_OFF[f"d{k}"]: PSUM_OFF[f"d{k}"] + 50]
                        nc.tensor.matmul(sl, lhsT=dk, rhs=Wc(k),
                                         start=False, stop=False)

                prev = None
                for bi in range(gb):
                    pmt = step1(bi)
                    if prev is not None:
                        step2(prev[0], prev[1], first_h=(prev[0] == 0))
                    prev = (bi, pmt)
                step2(prev[0], prev[1], first_h=(prev[0] == 0))
                # ---- -m I (fp16 identity moving; stop closes the h bank)
                nc.tensor.matmul(hps[:], lhsT=cI(0), rhs=it8[:, :W_],
                                 start=False, stop=True)

                s1b = sp_pool.tile([50, W_], FP16, tag="s1b")
                nc.scalar.copy(s1b[:], hps[:])

                # ---- powers: s2 = s*s, s3 = s*s2 (per-b) ----
                s2ps = pm.tile([50, W_], F32, tag="pmt")
                for bi in range(gb):
                    sl = slice(50 * bi, 50 * bi + 50)
                    nc.tensor.matmul(s2ps[:, sl], lhsT=s1b[:, sl], rhs=s1b[:, sl],
                                     start=True, stop=True)
                s2b = sp_pool.tile([50, W_], FP16, tag="s2b")
                nc.scalar.copy(s2b[:], s2ps[:])

                s3ps = pm.tile([50, W_], F32, tag="pmt")
                for bi in range(gb):
                    sl = slice(50 * bi, 50 * bi + 50)
                    nc.tensor.matmul(s3ps[:, sl], lhsT=s1b[:, sl], rhs=s2b[:, sl],
                                     start=True, stop=True)
                s3b = sp_pool.tile([50, W_], FP16, tag="s3b")
                nc.scalar.copy(s3b[:], s3ps[:])

                # ---- M2 = a7 s + a8 s2 + a6 I ----
                m2ps = pm.tile([50, W_], F32, tag="pmt")
                nc.tensor.matmul(m2ps[:], lhsT=cI(1), rhs=s1b[:], start=True, stop=False)
                nc.tensor.matmul(m2ps[:], lhsT=cI(2), rhs=s2b[:], start=False, stop=False)
                nc.tensor.matmul(m2ps[:], lhsT=cI(3), rhs=it8[:, :W_], start=False, stop=True)
                m2b = sp_pool.tile([50, W_], FP16, tag="m2b")
                nc.scalar.copy(m2b[:], m2ps[:])

                # ---- M1 = M2*s3 + a4 s + a5 s2 + a3 I ----
                m1ps = pm.tile([50, W_], F32, tag="pmt")
                nc.tensor.matmul(m1ps[:], lhsT=cI(4), rhs=s1b[:], start=True, stop=False)
                nc.tensor.matmul(m1ps[:], lhsT=cI(5), rhs=s2b[:], start=False, stop=False)
                nc.tensor.matmul(m1ps[:], lhsT=cI(6), rhs=it8[:, :W_], start=False, stop=True)
                for bi in range(gb):
                    sl = slice(50 * bi, 50 * bi + 50)
                    nc.tensor.matmul(m1ps[:, sl], lhsT=s3b[:, sl], rhs=m2b[:, sl],
                                     start=False, stop=False, skip_group_check=True)
                m1b = sp_pool.tile([50, W_], FP16, tag="m1b")
                nc.scalar.copy(m1b[:], m1ps[:])

                # ---- M0 = M1*s3 + a1 s + a2 s2  (a0 folded into host bias) ----
                m0ps = pm.tile([50, W_], F32, tag="pmt")
                nc.tensor.matmul(m0ps[:], lhsT=cI(7), rhs=s1b[:], start=True, stop=False)
                nc.tensor.matmul(m0ps[:], lhsT=cI(8), rhs=s2b[:], start=False, stop=True)
                for bi in range(gb):
                    sl = slice(50 * bi, 50 * bi + 50)
                    nc.tensor.matmul(m0ps[:, sl], lhsT=s3b[:, sl], rhs=m1b[:, sl],
                                     start=False, stop=False, skip_group_check=True)
                m0h = sp_pool.tile([50, W_], FP16, tag="m0h")
                nc.scalar.copy(m0h[:], m0ps[:])

                # ---- contraction: mul on GpSimd, X-reduce on DVE ----
                tmp = tp.tile([50, 7, gb, 50], FP16, tag="tmp")
                in0 = m0h[:].rearrange("p (b j) -> p b j", j=50)[:, None, :, :] \
                    .broadcast_to([50, 7, gb, 50])
                in1 = gt[:].rearrange("p (o j) -> p o j", j=50)[:, :, None, :] \
                    .broadcast_to([50, 7, gb, 50])
                nc.gpsimd.tensor_tensor(tmp[:], in0, in1, _mb.AluOpType.mult)
                red = rp.tile([50, 7 * gb], F32, tag="red")
                nc.vector.tensor_reduce(
                    red[:], tmp[:], axis=_mb.AxisListType.X, op=_mb.AluOpType.add,
                )
                nc.tensor.matmul(out_ps[:, out_off: out_off + 7 * gb],
                                 lhsT=on32[:], rhs=red[:], start=True, stop=True)

            off = 0
            for (b0, gb) in CHUNKS:
                do_group(b0, gb, off)
                off += 7 * gb

            o_sb = op_pool.tile([1, 7 * BC], F32, tag="osb")
            nc.scalar.copy(o_sb[:], out_ps[:])
            nc.sync.dma_start(out=o_d[:].rearrange("(a f) -> a f", a=1), in_=o_sb[:])

    _split_excess_waits(nc)
    return nc


def _get_program():
    if "nc" not in _CACHE:
        _apply_tile_patch()
        _CACHE["nc"] = _build_program()
    return _CACHE["nc"]


def _host_prep(W1, W2, W3, Wl, bl):
    W = (W1.astype(np.float64) @ W2.astype(np.float64) @ W3.astype(np.float64))
    Wstack = np.empty((100, 200), np.float16)
    for r in range(4):
        Wstack[:, 50 * r: 50 * r + 50] = W[100 * r: 100 * r + 100, :]

    iu, ju = np.triu_indices(N_OUT)
    G = np.zeros((7, N_OUT, N_OUT), np.float64)
    Wl64 = Wl.astype(np.float64)
    half = np.sqrt(2.0) / 2.0
    for k, (i, j) in enumerate(zip(iu, ju)):
        if i == j:
            G[:, i, j] = Wl64[:, k]
        else:
            G[:, i, j] = Wl64[:, k] * half
            G[:, j, i] = Wl64[:, k] * half
    # g tile [50, 350]: block o = G_o  (broadcast over the batch dim on device)
    gtile = np.empty((50, 350), np.float16)
    for o in range(7):
        gtile[:, 50 * o: 50 * o + 50] = G[o].astype(np.float16)

    a = np.array(COEF, np.float64)
    eye = np.eye(50, dtype=np.float32)
    ci = np.zeros((50, 450), np.float32)
    for k, cv in enumerate([-M_SHIFT, a[7], a[8], a[6], a[4], a[5], a[3], a[1], a[2]]):
        ci[:, 50 * k: 50 * k + 50] = np.float32(cv) * eye
    it8 = np.tile(eye, (1, 8)).astype(np.float16)

    bias = (bl.astype(np.float64) + a[0] * np.einsum("oii->o", G)).astype(np.float32)
    return Wstack, gtile, ci, it8, bias


def kernel(x, W1, W2, W3, Wl, bl):
    from concourse.bass_utils import run_bass_kernel_spmd

    x = np.asarray(x)
    W1, W2, W3 = np.asarray(W1), np.asarray(W2), np.asarray(W3)
    Wl, bl = np.asarray(Wl), np.asarray(bl)
    Wstack, gtile, ci, it8, bias = _host_prep(W1, W2, W3, Wl, bl)
    nc = _get_program()
    x = np.ascontiguousarray(x, np.float32)
    ones_col = np.ones((50, 1), np.float32)
    in_maps = [
        {"x": x[c * BC: (c + 1) * BC], "w": Wstack, "g": gtile, "ci": ci,
         "it": it8, "c32": ones_col}
        for c in range(N_CORES)
    ]
    res = run_bass_kernel_spmd(nc, in_maps, list(range(N_CORES)))
    outs = []
    for c in range(N_CORES):
        flat = res.results[c]["out"]  # chunked (o, bi) blocks per CHUNKS
        per_core = np.empty((BC, 7), np.float32)
        off = 0
        for (b0, gb) in CHUNKS:
            blk = flat[off: off + 7 * gb].reshape(7, gb)
            per_core[b0: b0 + gb] = blk.T
            off += 7 * gb
        outs.append(per_core)
    out = np.concatenate(outs, axis=0) + bias[None, :]
    return out.astype(np.float32)


if __name__ == "__main__":
    print("smoke build only")


# revision 14
# speedup vs baseline: 1.1794x; 1.1794x over previous
"""SPDNet kernel for Trainium2 (8 NeuronCores, data-parallel over batch).

Math: the reference's spd_rectify stages are identity maps (input SPD matrices
have all eigenvalues >= 1 >> EPS_RECT, and Stiefel compressions keep the
spectrum inside [lambda_min, lambda_max] subset of [1.37, 2.94]).  So the
network collapses to
    h_b   = W^T x_b W,         W = W1 @ W2 @ W3           (400x50, orthonormal)
    S_b   = logm(h_b)          (eigenvalues of h in [1.377, 2.937])
    out_b = <S_b, G_o> + bias  (G folds the sqrt(2)-scaled triu vectorization
                                and the final linear layer)
logm is evaluated eigendecomposition-free as a degree-8 polynomial in
s = h - m*I (near-minimax Chebyshev fit of log(m+s) on the padded spectrum
range [1.35, 2.96]; max fit error 1.2e-7), via Paterson-Stockmeyer with
v = s^3:  p(s) = (C2(s)*v + C1(s))*v + C0(s),  C_g quadratic in s.

x_b is SYMMETRIC, so only its lower block-triangle is DMA'd (100-row chunks
with column widths 128/200/300/400; the 128 pad keeps descriptor runs >=512B).
That cuts the dominant HBM traffic from 640KB to 411KB per matrix.  The cost
model charges a DMA on its ISSUING queue's engine (per-partition bytes x
0.3855ns, x2 under 512B runs, no shared DMA resource), so the x chunks are
spread greedily across the SP/Activation/DVE/Pool queues to run in parallel
with each other and with compute.  h is then assembled without ever
materializing W^T x:
    per 100x100 block (r>=c):  P_rc = X_rc^T W_r      (x stationary, fp16 W
                                                       moving, 50 rows each)
    accumulate Psum_c = sum_{r>c} P_rc in PSUM, evict bank to fp16, then
    h = sum_c [Psum_c^T W_c + W_c^T Psum_c] + sum_k P_kk^T W_k - m I
(all step-2 matmuls are fp16 x fp16, 1 cycle/row at any width).  The
polynomial runs with fp16 power/stationary tiles and fp16-moving combines.
Final contraction <S_b, G_o>: elementwise mul on GpSimd, X-reduce on DVE,
partition-sum via ones-vector matmul on the tensor engine.
"""

import numpy as np

N_CORES = 8
B_FULL = 256
BC = B_FULL // N_CORES      # 32 per core
N_IN = 400
N_OUT = 50

# log(m + s) polynomial on s in [lo-m, hi-m], from Chebyshev interpolation
# (degree 8, domain [1.35, 2.96]); coefficients are monomial-basis in s.
M_SHIFT = 2.1550000000000002
COEF = [
    0.7677907235557108, 0.4640362223750899, -0.10766484774906421,
    0.03332547763901113, -0.011599509906866342, 0.004203545486868787,
    -0.0016222327568142045, 0.0008559664117230024, -0.0003500826285455622,
]

# lower-triangle row-chunk DMA widths (>=128 cols keeps runs >= 512B)
W_R = [128, 200, 300, 400]

# batch chunks (start, size); small last chunk shortens the serial tail
CHUNKS = [(0, 6), (6, 8), (14, 8), (22, 6), (28, 4)]

# per-queue fixed compute load estimates (ns) used by the greedy DMA spread
DMA_CYC = 0.3855          # ns per per-partition byte (v1 cost model)
FIXED_LOAD = {"SP": 0.0, "ACT": 15500.0, "DVE": 12500.0, "POOL": 12500.0}
EVICT_NS = 460.0          # per P-bank eviction estimate

# P-bank region offsets: Psum_c (c=0,1,2) then P_kk (k=0..3)
PSUM_OFF = {"acc0": 0, "acc1": 50, "acc2": 100,
            "d0": 150, "d1": 200, "d2": 250, "d3": 300}
PBANK_W = 350

CFG = {"xp": 2, "pmp": 4, "sp": 3, "tp": 2, "rp": 2,
       "pb": 2, "ph": 2, "pm": 3}

_CACHE = {}


def _apply_tile_patch():
    """This container's walrus rejects instructions carrying more than a
    couple of semaphore waits ("Too many sync wait commands") which the Tile
    tail drain always does.  Split the drain's waits across one sync-engine
    nop per logical processor instead."""
    if _CACHE.get("patched"):
        return
    import concourse.tile as ctile
    from bass_rust import VectorClock, ScopedClock, N_PROCS

    def _drain_and_barrier_split(self, tick_clock, wait_clock):
        gc = tick_clock.global_clock
        for p in range(N_PROCS):
            if gc[p] == 0:
                continue
            sub = [gc[q] if q == p else 0 for q in range(N_PROCS)]
            nop_inst = self.nc.sync.nop(nofuse=True, hint=f"drain_split_{p}")
            wait_clock.add_sem_waits(
                nop_inst.ins, ScopedClock({None: VectorClock(sub)})
            )
        self.nc.sync.drain()  # waits already emitted on the nops above
        self.nc.all_engine_barrier()
        assert self.sems is not None
        popped = self.nc._tile_sem_poison_stack.pop()
        assert popped is self._sem_poison
        self.nc.clear_and_free_semaphores(list(self.sems.allocated().values()))
        self.nc.all_engine_barrier()

    ctile.TileContext._drain_and_barrier = _drain_and_barrier_split
    _CACHE["patched"] = True


def _split_excess_waits(nc, limit=1):
    """This container's walrus rejects instructions with more than `limit`
    semaphore waits.  Move excess waits onto same-engine nops inserted
    immediately before the instruction (identical stall semantics)."""
    import concourse.mybir as mybir

    n_split = 0
    for fn in nc.m.functions:
        for blk in fn.blocks:
            new_insts = []
            for inst in blk.instructions:
                si = getattr(inst, "sync_info", None)
                waits = list(si.on_wait) if si is not None and si.on_wait else []
                if len(waits) > limit:
                    extra, keep = waits[:-limit], waits[-limit:]
                    for ci, cs in enumerate(range(0, len(extra), limit)):
                        chunk = extra[cs: cs + limit]
                        nop = mybir.InstNoOp(
                            name=f"{inst.name}-ws{ci}", ins=[], outs=[]
                        )
                        nop.engine = inst.engine
                        nop.sync_info = mybir.SyncInfo(on_wait=chunk, on_update=[])
                        new_insts.append(nop)
                        n_split += 1
                    si.on_wait = keep
                new_insts.append(inst)
            if n_split:
                blk.instructions[:] = new_insts
    return n_split


def _build_program():
    import concourse.bass as bass
    import concourse.mybir as mybir
    from concourse import tile

    F32 = mybir.dt.float32
    F32R = mybir.dt.float32r
    FP16 = mybir.dt.float16
    nc = bass.Bass()
    x_d = nc.declare_dram_parameter("x", [BC, N_IN, N_IN], F32R, isOutput=False)
    w_d = nc.declare_dram_parameter("w", [100, 200], FP16, isOutput=False)
    g_d = nc.declare_dram_parameter("g", [50, 350], FP16, isOutput=False)
    ci_d = nc.declare_dram_parameter("ci", [50, 450], F32R, isOutput=False)
    it_d = nc.declare_dram_parameter("it", [50, 400], FP16, isOutput=False)
    c32_d = nc.declare_dram_parameter("c32", [50, 1], F32, isOutput=False)
    o_d = nc.declare_dram_parameter("out", [7 * BC], F32, isOutput=True)

    # ---- greedy spread of x-chunk DMAs + P-bank evictions over queues ----
    load = dict(FIXED_LOAD)
    jobs = []  # (cost, kind, group, r)
    for gi, (b0, gb) in enumerate(CHUNKS):
        for r in range(4):
            jobs.append((gb * W_R[r] * 4 * DMA_CYC, "dma", gi, r))
        jobs.append((gb * EVICT_NS, "ev", gi, -1))
    jobs.sort(key=lambda j: -j[0])
    dma_q = {}
    ev_q = {}
    for cost, kind, gi, r in jobs:
        cands = ("SP", "ACT", "POOL") if kind == "dma" else ("ACT", "DVE", "POOL")
        best = min(cands, key=lambda q: load[q])
        load[best] += cost
        if kind == "dma":
            dma_q[(gi, r)] = best
        else:
            ev_q[gi] = best

    with tile.TileContext(nc) as tc:
        with (
            tc.tile_pool(name="const", bufs=1) as constp,
            tc.tile_pool(name="xp", bufs=CFG["xp"]) as xp,
            tc.tile_pool(name="pmp", bufs=CFG["pmp"]) as pmp,
            tc.tile_pool(name="sp", bufs=CFG["sp"]) as sp_pool,
            tc.tile_pool(name="tp", bufs=CFG["tp"]) as tp,
            tc.tile_pool(name="rp", bufs=CFG["rp"]) as rp,
            tc.tile_pool(name="op", bufs=1) as op_pool,
            tc.tile_pool(name="pb", bufs=CFG["pb"], space="PSUM") as pb,
            tc.tile_pool(name="ph", bufs=CFG["ph"], space="PSUM") as ph,
            tc.tile_pool(name="pm", bufs=CFG["pm"], space="PSUM") as pm,
            tc.tile_pool(name="pr", bufs=1, space="PSUM") as pr,
        ):
            QUEUE = {"SP": nc.sync, "ACT": nc.scalar, "DVE": nc.vector,
                     "POOL": nc.gpsimd}
            COPY = {"ACT": nc.scalar.copy, "DVE": nc.vector.tensor_copy,
                    "POOL": nc.gpsimd.tensor_copy}

            wh = constp.tile([100, 200], FP16, tag="wh")
            nc.sync.dma_start(out=wh[:], in_=w_d[:])
            ci_t = constp.tile([50, 450], F32R, tag="ci")
            nc.gpsimd.dma_start(out=ci_t[:], in_=ci_d[:])
            it8 = constp.tile([50, 400], FP16, tag="it8")
            nc.gpsimd.dma_start(out=it8[:], in_=it_d[:])
            gt = constp.tile([50, 350], FP16, tag="gt")
            nc.scalar.dma_start(out=gt[:], in_=g_d[:])
            on32 = constp.tile([50, 1], F32, tag="on32")
            nc.sync.dma_start(out=on32[:], in_=c32_d[:])

            cI = lambda k: ci_t[:, 50 * k: 50 * k + 50]
            # blocks: 0:-m, 1:a7, 2:a8, 3:a6, 4:a4, 5:a5, 6:a3, 7:a1, 8:a2
            Wc = lambda r: wh[:, 50 * r: 50 * r + 50]

            out_ps = pr.tile([1, 7 * BC], F32, tag="ops")
            import concourse.mybir as _mb

            def emit_xdma(gi):
                b0, gb = CHUNKS[gi]
                xts = []
                for r in range(4):
                    w = W_R[r]
                    xt = xp.tile([100, gb, w], F32R, tag=f"x{r}")
                    QUEUE[dma_q[(gi, r)]].dma_start(
                        out=xt[:],
                        in_=x_d[b0: b0 + gb, 100 * r: 100 * r + 100, 0:w]
                        .rearrange("b p j -> p b j"),
                    )
                    xts.append(xt)
                return xts

            def do_group(gi, xts, out_off):
                b0, gb = CHUNKS[gi]
                W_ = 50 * gb
                evict = COPY[ev_q[gi]]

                hps = ph.tile([50, W_], F32, tag="h")

                def step1(bi):
                    pb_t = pb.tile([100, PBANK_W], F32, tag="pbk")
                    first = True
                    for r in range(4):
                        for c in range(r + 1):
                            off = PSUM_OFF[f"d{r}"] if c == r else PSUM_OFF[f"acc{c}"]
                            nc.tensor.matmul(
                                pb_t[:, off: off + 50],
                                lhsT=xts[r][:, bi, 100 * c: 100 * c + 100],
                                rhs=Wc(r),
                                start=first, stop=(r == 3 and c == 3),
                            )
                            first = False
                    pmt = pmp.tile([100, PBANK_W], FP16, tag="pmt")
                    evict(pmt[:], pb_t[:])
                    return pmt

                def step2(bi, pmt, first_h):
                    sl = hps[:, 50 * bi: 50 * bi + 50]
                    mm = 0
                    for c in range(3):
                        acc = pmt[:, PSUM_OFF[f"acc{c}"]: PSUM_OFF[f"acc{c}"] + 50]
                        nc.tensor.matmul(sl, lhsT=Wc(c), rhs=acc,
                                         start=(first_h and mm == 0), stop=False)
                        mm += 1
                        nc.tensor.matmul(sl, lhsT=acc, rhs=Wc(c),
                                         start=False, stop=False)
                        mm += 1
                    for k in range(4):
                        dk = pmt[:, PSUM_OFF[f"d{k}"]: PSUM_OFF[f"d{k}"] + 50]
                        nc.tensor.matmul(sl, lhsT=dk, rhs=Wc(k),
                                         start=False, stop=False)

                prev = None
                for bi in range(gb):
                    pmt = step1(bi)
                    if prev is not None:
                        step2(prev[0], prev[1], first_h=(prev[0] == 0))
                    prev = (bi, pmt)
                step2(prev[0], prev[1], first_h=(prev[0] == 0))
                # ---- -m I (fp16 identity moving; stop closes the h bank)
                nc.tensor.matmul(hps[:], lhsT=cI(0), rhs=it8[:, :W_],
                                 start=False, stop=True)

                s1b = sp_pool.tile([50, W_], FP16, tag="s1b")
                nc.scalar.copy(s1b[:], hps[:])

                # ---- powers: s2 = s*s, s3 = s*s2 (per-b) ----
                s2ps = pm.tile([50, W_], F32, tag="pmt")
                for bi in range(gb):
                    sl = slice(50 * bi, 50 * bi + 50)
                    nc.tensor.matmul(s2ps[:, sl], lhsT=s1b[:, sl], rhs=s1b[:, sl],
                                     start=True, stop=True)
                s2b = sp_pool.tile([50, W_], FP16, tag="s2b")
                nc.scalar.copy(s2b[:], s2ps[:])

                s3ps = pm.tile([50, W_], F32, tag="pmt")
                for bi in range(gb):
                    sl = slice(50 * bi, 50 * bi + 50)
                    nc.tensor.matmul(s3ps[:, sl], lhsT=s1b[:, sl], rhs=s2b[:, sl],
                                     start=True, stop=True)
                s3b = sp_pool.tile([50, W_], FP16, tag="s3b")
                nc.scalar.copy(s3b[:], s3ps[:])

                # ---- M2 = a7 s + a8 s2 + a6 I ----
                m2ps = pm.tile([50, W_], F32, tag="pmt")
                nc.tensor.matmul(m2ps[:], lhsT=cI(1), rhs=s1b[:], start=True, stop=False)
                nc.tensor.matmul(m2ps[:], lhsT=cI(2), rhs=s2b[:], start=False, stop=False)
                nc.tensor.matmul(m2ps[:], lhsT=cI(3), rhs=it8[:, :W_], start=False, stop=True)
                m2b = sp_pool.tile([50, W_], FP16, tag="m2b")
                nc.scalar.copy(m2b[:], m2ps[:])

                # ---- M1 = M2*s3 + a4 s + a5 s2 + a3 I ----
                m1ps = pm.tile([50, W_], F32, tag="pmt")
                nc.tensor.matmul(m1ps[:], lhsT=cI(4), rhs=s1b[:], start=True, stop=False)
                nc.tensor.matmul(m1ps[:], lhsT=cI(5), rhs=s2b[:], start=False, stop=False)
                nc.tensor.matmul(m1ps[:], lhsT=cI(6), rhs=it8[:, :W_], start=False, stop=True)
                for bi in range(gb):
                    sl = slice(50 * bi, 50 * bi + 50)
                    nc.tensor.matmul(m1ps[:, sl], lhsT=s3b[:, sl], rhs=m2b[:, sl],
                                     start=False, stop=False, skip_group_check=True)
                m1b = sp_pool.tile([50, W_], FP16, tag="m1b")
                nc.scalar.copy(m1b[:], m1ps[:])

                # ---- M0 = M1*s3 + a1 s + a2 s2  (a0 folded into host bias) ----
                m0ps = pm.tile([50, W_], F32, tag="pmt")
                nc.tensor.matmul(m0ps[:], lhsT=cI(7), rhs=s1b[:], start=True, stop=False)
                nc.tensor.matmul(m0ps[:], lhsT=cI(8), rhs=s2b[:], start=False, stop=True)
                for bi in range(gb):
                    sl = slice(50 * bi, 50 * bi + 50)
                    nc.tensor.matmul(m0ps[:, sl], lhsT=s3b[:, sl], rhs=m1b[:, sl],
                                     start=False, stop=False, skip_group_check=True)
                m0h = sp_pool.tile([50, W_], FP16, tag="m0h")
                nc.scalar.copy(m0h[:], m0ps[:])

                # ---- contraction: mul on GpSimd, X-reduce on DVE ----
                tmp = tp.tile([50, 7, gb, 50], FP16, tag="tmp")
                in0 = m0h[:].rearrange("p (b j) -> p b j", j=50)[:, None, :, :] \
                    .broadcast_to([50, 7, gb, 50])
                in1 = gt[:].rearrange("p (o j) -> p o j", j=50)[:, :, None, :] \
                    .broadcast_to([50, 7, gb, 50])
                nc.gpsimd.tensor_tensor(tmp[:], in0, in1, _mb.AluOpType.mult)
                red = rp.tile([50, 7 * gb], F32, tag="red")
                nc.vector.tensor_reduce(
                    red[:], tmp[:], axis=_mb.AxisListType.X, op=_mb.AluOpType.add,
                )
                nc.tensor.matmul(out_ps[:, out_off: out_off + 7 * gb],
                                 lhsT=on32[:], rhs=red[:], start=True, stop=True)

            off = 0
            xts = emit_xdma(0)
            for gi, (b0, gb) in enumerate(CHUNKS):
                nxt = emit_xdma(gi + 1) if gi + 1 < len(CHUNKS) else None
                do_group(gi, xts, off)
                xts = nxt
                off += 7 * gb

            o_sb = op_pool.tile([1, 7 * BC], F32, tag="osb")
            nc.scalar.copy(o_sb[:], out_ps[:])
            nc.sync.dma_start(out=o_d[:].rearrange("(a f) -> a f", a=1), in_=o_sb[:])

    _split_excess_waits(nc)
    return nc


def _get_program():
    if "nc" not in _CACHE:
        _apply_tile_patch()
        _CACHE["nc"] = _build_program()
    return _CACHE["nc"]


def _host_prep(W1, W2, W3, Wl, bl):
    W = (W1.astype(np.float64) @ W2.astype(np.float64) @ W3.astype(np.float64))
    Wstack = np.empty((100, 200), np.float16)
    for r in range(4):
        Wstack[:, 50 * r: 50 * r + 50] = W[100 * r: 100 * r + 100, :]

    iu, ju = np.triu_indices(N_OUT)
    G = np.zeros((7, N_OUT, N_OUT), np.float64)
    Wl64 = Wl.astype(np.float64)
    half = np.sqrt(2.0) / 2.0
    for k, (i, j) in enumerate(zip(iu, ju)):
        if i == j:
            G[:, i, j] = Wl64[:, k]
        else:
            G[:, i, j] = Wl64[:, k] * half
            G[:, j, i] = Wl64[:, k] * half
    # g tile [50, 350]: block o = G_o  (broadcast over the batch dim on device)
    gtile = np.empty((50, 350), np.float16)
    for o in range(7):
        gtile[:, 50 * o: 50 * o + 50] = G[o].astype(np.float16)

    a = np.array(COEF, np.float64)
    eye = np.eye(50, dtype=np.float32)
    ci = np.zeros((50, 450), np.float32)
    for k, cv in enumerate([-M_SHIFT, a[7], a[8], a[6], a[4], a[5], a[3], a[1], a[2]]):
        ci[:, 50 * k: 50 * k + 50] = np.float32(cv) * eye
    it8 = np.tile(eye, (1, 8)).astype(np.float16)

    bias = (bl.astype(np.float64) + a[0] * np.einsum("oii->o", G)).astype(np.float32)
    return Wstack, gtile, ci, it8, bias


def kernel(x, W1, W2, W3, Wl, bl):
    from concourse.bass_utils import run_bass_kernel_spmd

    x = np.asarray(x)
    W1, W2, W3 = np.asarray(W1), np.asarray(W2), np.asarray(W3)
    Wl, bl = np.asarray(Wl), np.asarray(bl)
    Wstack, gtile, ci, it8, bias = _host_prep(W1, W2, W3, Wl, bl)
    nc = _get_program()
    x = np.ascontiguousarray(x, np.float32)
    ones_col = np.ones((50, 1), np.float32)
    in_maps = [
        {"x": x[c * BC: (c + 1) * BC], "w": Wstack, "g": gtile, "ci": ci,
         "it": it8, "c32": ones_col}
        for c in range(N_CORES)
    ]
    res = run_bass_kernel_spmd(nc, in_maps, list(range(N_CORES)))
    outs = []
    for c in range(N_CORES):
        flat = res.results[c]["out"]  # chunked (o, bi) blocks per CHUNKS
        per_core = np.empty((BC, 7), np.float32)
        off = 0
        for (b0, gb) in CHUNKS:
            blk = flat[off: off + 7 * gb].reshape(7, gb)
            per_core[b0: b0 + gb] = blk.T
            off += 7 * gb
        outs.append(per_core)
    out = np.concatenate(outs, axis=0) + bias[None, :]
    return out.astype(np.float32)


if __name__ == "__main__":
    print("smoke build only")


# revision 18
# speedup vs baseline: 1.2543x; 1.0635x over previous
"""SPDNet kernel for Trainium2 (8 NeuronCores, data-parallel over batch).

Math: the reference's spd_rectify stages are identity maps (input SPD matrices
have all eigenvalues >= 1 >> EPS_RECT, and Stiefel compressions keep the
spectrum inside [lambda_min, lambda_max] subset of [1.37, 2.94]).  So the
network collapses to
    h_b   = W^T x_b W,         W = W1 @ W2 @ W3           (400x50, orthonormal)
    S_b   = logm(h_b)          (eigenvalues of h in [1.377, 2.937])
    out_b = <S_b, G_o> + bias  (G folds the sqrt(2)-scaled triu vectorization
                                and the final linear layer)
logm is evaluated eigendecomposition-free as a degree-8 polynomial in
s = h - m*I (near-minimax Chebyshev fit of log(m+s) on the padded spectrum
range [1.35, 2.96]; max fit error 1.2e-7), via Paterson-Stockmeyer with
v = s^3:  p(s) = (C2(s)*v + C1(s))*v + C0(s),  C_g quadratic in s.

x_b is SYMMETRIC, so only its lower block-triangle is DMA'd (100-row chunks
with column widths 128/200/300/400; the 128 pad keeps descriptor runs >=512B).
That cuts the dominant HBM traffic from 640KB to 411KB per matrix.  The cost
model charges a DMA on its ISSUING queue's engine (per-partition bytes x
0.3855ns, x2 under 512B runs, no shared DMA resource), so the x chunks are
spread greedily across the SP/Activation/DVE/Pool queues to run in parallel
with each other and with compute.  h is then assembled without ever
materializing W^T x:
    per 100x100 block (r>=c):  P_rc = X_rc^T W_r      (x stationary, fp16 W
                                                       moving, 50 rows each)
    accumulate Psum_c = sum_{r>c} P_rc in PSUM, evict bank to fp16, then
    h = sum_c [Psum_c^T W_c + W_c^T Psum_c] + sum_k P_kk^T W_k - m I
(all step-2 matmuls are fp16 x fp16, 1 cycle/row at any width).  The
polynomial runs with fp16 power/stationary tiles and fp16-moving combines.
Final contraction <S_b, G_o>: elementwise mul on GpSimd, X-reduce on DVE,
partition-sum via ones-vector matmul on the tensor engine.
"""

import numpy as np

N_CORES = 8
B_FULL = 256
BC = B_FULL // N_CORES      # 32 per core
N_IN = 400
N_OUT = 50

# log(m + s) polynomial on s in [lo-m, hi-m], from Chebyshev interpolation
# (degree 8, domain [1.35, 2.96]); coefficients are monomial-basis in s.
M_SHIFT = 2.1550000000000002
COEF = [
    0.7677907235557108, 0.4640362223750899, -0.10766484774906421,
    0.03332547763901113, -0.011599509906866342, 0.004203545486868787,
    -0.0016222327568142045, 0.0008559664117230024, -0.0003500826285455622,
]

# lower-triangle row-chunk DMA widths (>=128 cols keeps runs >= 512B)
W_R = [128, 200, 300, 400]

# batch chunks (start, size); small last chunk shortens the serial tail
CHUNKS = [(0, 6), (6, 8), (14, 8), (22, 6), (28, 4)]

# per-queue fixed compute load estimates (ns) used by the greedy DMA spread
DMA_CYC = 0.3855          # ns per per-partition byte (v1 cost model)
FIXED_LOAD = {"SP": 0.0, "ACT": 15500.0, "DVE": 12500.0, "POOL": 12500.0}
EVICT_NS = 460.0          # per P-bank eviction estimate

# P-bank region offsets: Psum_c (c=0,1,2) then P_kk (k=0..3)
PSUM_OFF = {"acc0": 0, "acc1": 50, "acc2": 100,
            "d0": 150, "d1": 200, "d2": 250, "d3": 300}
PBANK_W = 350

CFG = {"xp": 3, "pmp": 6, "sp": 3, "tp": 2, "rp": 2,
       "pb": 2, "ph": 2, "pm": 3}

_CACHE = {}


def _apply_tile_patch():
    """This container's walrus rejects instructions carrying more than a
    couple of semaphore waits ("Too many sync wait commands") which the Tile
    tail drain always does.  Split the drain's waits across one sync-engine
    nop per logical processor instead."""
    if _CACHE.get("patched"):
        return
    import concourse.tile as ctile
    from bass_rust import VectorClock, ScopedClock, N_PROCS

    def _drain_and_barrier_split(self, tick_clock, wait_clock):
        gc = tick_clock.global_clock
        for p in range(N_PROCS):
            if gc[p] == 0:
                continue
            sub = [gc[q] if q == p else 0 for q in range(N_PROCS)]
            nop_inst = self.nc.sync.nop(nofuse=True, hint=f"drain_split_{p}")
            wait_clock.add_sem_waits(
                nop_inst.ins, ScopedClock({None: VectorClock(sub)})
            )
        self.nc.sync.drain()  # waits already emitted on the nops above
        self.nc.all_engine_barrier()
        assert self.sems is not None
        popped = self.nc._tile_sem_poison_stack.pop()
        assert popped is self._sem_poison
        self.nc.clear_and_free_semaphores(list(self.sems.allocated().values()))
        self.nc.all_engine_barrier()

    ctile.TileContext._drain_and_barrier = _drain_and_barrier_split
    _CACHE["patched"] = True


def _split_excess_waits(nc, limit=1):
    """This container's walrus rejects instructions with more than `limit`
    semaphore waits.  Move excess waits onto same-engine nops inserted
    immediately before the instruction (identical stall semantics)."""
    import concourse.mybir as mybir

    n_split = 0
    for fn in nc.m.functions:
        for blk in fn.blocks:
            new_insts = []
            for inst in blk.instructions:
                si = getattr(inst, "sync_info", None)
                waits = list(si.on_wait) if si is not None and si.on_wait else []
                if len(waits) > limit:
                    extra, keep = waits[:-limit], waits[-limit:]
                    for ci, cs in enumerate(range(0, len(extra), limit)):
                        chunk = extra[cs: cs + limit]
                        nop = mybir.InstNoOp(
                            name=f"{inst.name}-ws{ci}", ins=[], outs=[]
                        )
                        nop.engine = inst.engine
                        nop.sync_info = mybir.SyncInfo(on_wait=chunk, on_update=[])
                        new_insts.append(nop)
                        n_split += 1
                    si.on_wait = keep
                new_insts.append(inst)
            if n_split:
                blk.instructions[:] = new_insts
    return n_split


def _build_program():
    import concourse.bass as bass
    import concourse.mybir as mybir
    from concourse import tile

    F32 = mybir.dt.float32
    F32R = mybir.dt.float32r
    FP16 = mybir.dt.float16
    nc = bass.Bass()
    x_d = nc.declare_dram_parameter("x", [BC, N_IN, N_IN], F32R, isOutput=False)
    w_d = nc.declare_dram_parameter("w", [100, 200], FP16, isOutput=False)
    g_d = nc.declare_dram_parameter("g", [50, 350], FP16, isOutput=False)
    ci_d = nc.declare_dram_parameter("ci", [50, 450], F32R, isOutput=False)
    it_d = nc.declare_dram_parameter("it", [50, 400], FP16, isOutput=False)
    c32_d = nc.declare_dram_parameter("c32", [50, 1], F32, isOutput=False)
    o_d = nc.declare_dram_parameter("out", [7 * BC], F32, isOutput=True)

    # ---- greedy spread of x-chunk DMAs + P-bank evictions over queues ----
    load = dict(FIXED_LOAD)
    jobs = []  # (cost, kind, group, r)
    for gi, (b0, gb) in enumerate(CHUNKS):
        for r in range(4):
            jobs.append((gb * W_R[r] * 4 * DMA_CYC, "dma", gi, r))
        jobs.append((gb * EVICT_NS, "ev", gi, -1))
    jobs.sort(key=lambda j: -j[0])
    dma_q = {}
    ev_q = {}
    for cost, kind, gi, r in jobs:
        cands = ("SP", "ACT", "POOL") if kind == "dma" else ("ACT", "DVE", "POOL")
        best = min(cands, key=lambda q: load[q])
        load[best] += cost
        if kind == "dma":
            dma_q[(gi, r)] = best
        else:
            ev_q[gi] = best

    with tile.TileContext(nc) as tc:
        with (
            tc.tile_pool(name="const", bufs=1) as constp,
            tc.tile_pool(name="xp", bufs=CFG["xp"]) as xp,
            tc.tile_pool(name="pmp", bufs=CFG["pmp"]) as pmp,
            tc.tile_pool(name="sp", bufs=CFG["sp"]) as sp_pool,
            tc.tile_pool(name="tp", bufs=CFG["tp"]) as tp,
            tc.tile_pool(name="rp", bufs=CFG["rp"]) as rp,
            tc.tile_pool(name="op", bufs=1) as op_pool,
            tc.tile_pool(name="pb", bufs=CFG["pb"], space="PSUM") as pb,
            tc.tile_pool(name="ph", bufs=CFG["ph"], space="PSUM") as ph,
            tc.tile_pool(name="pm", bufs=CFG["pm"], space="PSUM") as pm,
            tc.tile_pool(name="pr", bufs=1, space="PSUM") as pr,
        ):
            QUEUE = {"SP": nc.sync, "ACT": nc.scalar, "DVE": nc.vector,
                     "POOL": nc.gpsimd}
            COPY = {"ACT": nc.scalar.copy, "DVE": nc.vector.tensor_copy,
                    "POOL": nc.gpsimd.tensor_copy}

            wh = constp.tile([100, 200], FP16, tag="wh")
            nc.sync.dma_start(out=wh[:], in_=w_d[:])
            ci_t = constp.tile([50, 450], F32R, tag="ci")
            nc.gpsimd.dma_start(out=ci_t[:], in_=ci_d[:])
            it8 = constp.tile([50, 400], FP16, tag="it8")
            nc.gpsimd.dma_start(out=it8[:], in_=it_d[:])
            gt = constp.tile([50, 350], FP16, tag="gt")
            nc.scalar.dma_start(out=gt[:], in_=g_d[:])
            on32 = constp.tile([50, 1], F32, tag="on32")
            nc.sync.dma_start(out=on32[:], in_=c32_d[:])

            cI = lambda k: ci_t[:, 50 * k: 50 * k + 50]
            # blocks: 0:-m, 1:a7, 2:a8, 3:a6, 4:a4, 5:a5, 6:a3, 7:a1, 8:a2
            Wc = lambda r: wh[:, 50 * r: 50 * r + 50]

            out_ps = pr.tile([1, 7 * BC], F32, tag="ops")
            import concourse.mybir as _mb

            def emit_xdma(gi):
                b0, gb = CHUNKS[gi]
                xts = []
                for r in range(4):
                    w = W_R[r]
                    xt = xp.tile([100, gb, w], F32R, tag=f"x{r}")
                    QUEUE[dma_q[(gi, r)]].dma_start(
                        out=xt[:],
                        in_=x_d[b0: b0 + gb, 100 * r: 100 * r + 100, 0:w]
                        .rearrange("b p j -> p b j"),
                    )
                    xts.append(xt)
                return xts

            def do_groupA(gi, xts):
                """step1 + step2 + (-mI): produce the h PSUM tile."""
                b0, gb = CHUNKS[gi]
                W_ = 50 * gb
                evict = COPY[ev_q[gi]]

                hps = ph.tile([50, W_], F32, tag="h")

                def step1(bi):
                    pb_t = pb.tile([100, PBANK_W], F32, tag="pbk")
                    first = True
                    for r in range(4):
                        for c in range(r + 1):
                            off = PSUM_OFF[f"d{r}"] if c == r else PSUM_OFF[f"acc{c}"]
                            nc.tensor.matmul(
                                pb_t[:, off: off + 50],
                                lhsT=xts[r][:, bi, 100 * c: 100 * c + 100],
                                rhs=Wc(r),
                                start=first, stop=(r == 3 and c == 3),
                            )
                            first = False
                    pmt = pmp.tile([100, PBANK_W], FP16, tag="pmt")
                    evict(pmt[:], pb_t[:])
                    return pmt

                def step2(bi, pmt, first_h):
                    sl = hps[:, 50 * bi: 50 * bi + 50]
                    mm = 0
                    for c in range(3):
                        acc = pmt[:, PSUM_OFF[f"acc{c}"]: PSUM_OFF[f"acc{c}"] + 50]
                        nc.tensor.matmul(sl, lhsT=Wc(c), rhs=acc,
                                         start=(first_h and mm == 0), stop=False)
                        mm += 1
                        nc.tensor.matmul(sl, lhsT=acc, rhs=Wc(c),
                                         start=False, stop=False)
                        mm += 1
                    for k in range(4):
                        dk = pmt[:, PSUM_OFF[f"d{k}"]: PSUM_OFF[f"d{k}"] + 50]
                        nc.tensor.matmul(sl, lhsT=dk, rhs=Wc(k),
                                         start=False, stop=False)

                prev = None
                for bi in range(gb):
                    pmt = step1(bi)
                    if prev is not None:
                        step2(prev[0], prev[1], first_h=(prev[0] == 0))
                    prev = (bi, pmt)
                step2(prev[0], prev[1], first_h=(prev[0] == 0))
                # ---- -m I (fp16 identity moving; stop closes the h bank)
                nc.tensor.matmul(hps[:], lhsT=cI(0), rhs=it8[:, :W_],
                                 start=False, stop=True)
                return hps

            def do_groupB(gi, hps, out_off):
                """logm polynomial + contraction + output accumulation."""
                b0, gb = CHUNKS[gi]
                W_ = 50 * gb

                s1b = sp_pool.tile([50, W_], FP16, tag="s1b")
                nc.scalar.copy(s1b[:], hps[:])

                # ---- powers: s2 = s*s, s3 = s*s2 (per-b) ----
                s2ps = pm.tile([50, W_], F32, tag="pmt")
                for bi in range(gb):
                    sl = slice(50 * bi, 50 * bi + 50)
                    nc.tensor.matmul(s2ps[:, sl], lhsT=s1b[:, sl], rhs=s1b[:, sl],
                                     start=True, stop=True)
                s2b = sp_pool.tile([50, W_], FP16, tag="s2b")
                nc.scalar.copy(s2b[:], s2ps[:])

                s3ps = pm.tile([50, W_], F32, tag="pmt")
                for bi in range(gb):
                    sl = slice(50 * bi, 50 * bi + 50)
                    nc.tensor.matmul(s3ps[:, sl], lhsT=s1b[:, sl], rhs=s2b[:, sl],
                                     start=True, stop=True)
                s3b = sp_pool.tile([50, W_], FP16, tag="s3b")
                nc.scalar.copy(s3b[:], s3ps[:])

                # ---- M2 = a7 s + a8 s2 + a6 I ----
                m2ps = pm.tile([50, W_], F32, tag="pmt")
                nc.tensor.matmul(m2ps[:], lhsT=cI(1), rhs=s1b[:], start=True, stop=False)
                nc.tensor.matmul(m2ps[:], lhsT=cI(2), rhs=s2b[:], start=False, stop=False)
                nc.tensor.matmul(m2ps[:], lhsT=cI(3), rhs=it8[:, :W_], start=False, stop=True)
                m2b = sp_pool.tile([50, W_], FP16, tag="m2b")
                nc.scalar.copy(m2b[:], m2ps[:])

                # ---- M1 = M2*s3 + a4 s + a5 s2 + a3 I ----
                m1ps = pm.tile([50, W_], F32, tag="pmt")
                nc.tensor.matmul(m1ps[:], lhsT=cI(4), rhs=s1b[:], start=True, stop=False)
                nc.tensor.matmul(m1ps[:], lhsT=cI(5), rhs=s2b[:], start=False, stop=False)
                nc.tensor.matmul(m1ps[:], lhsT=cI(6), rhs=it8[:, :W_], start=False, stop=True)
                for bi in range(gb):
                    sl = slice(50 * bi, 50 * bi + 50)
                    nc.tensor.matmul(m1ps[:, sl], lhsT=s3b[:, sl], rhs=m2b[:, sl],
                                     start=False, stop=False, skip_group_check=True)
                m1b = sp_pool.tile([50, W_], FP16, tag="m1b")
                nc.scalar.copy(m1b[:], m1ps[:])

                # ---- M0 = M1*s3 + a1 s + a2 s2  (a0 folded into host bias) ----
                m0ps = pm.tile([50, W_], F32, tag="pmt")
                nc.tensor.matmul(m0ps[:], lhsT=cI(7), rhs=s1b[:], start=True, stop=False)
                nc.tensor.matmul(m0ps[:], lhsT=cI(8), rhs=s2b[:], start=False, stop=True)
                for bi in range(gb):
                    sl = slice(50 * bi, 50 * bi + 50)
                    nc.tensor.matmul(m0ps[:, sl], lhsT=s3b[:, sl], rhs=m1b[:, sl],
                                     start=False, stop=False, skip_group_check=True)
                m0h = sp_pool.tile([50, W_], FP16, tag="m0h")
                nc.scalar.copy(m0h[:], m0ps[:])

                # ---- contraction: mul on GpSimd, X-reduce on DVE ----
                tmp = tp.tile([50, 7, gb, 50], FP16, tag="tmp")
                in0 = m0h[:].rearrange("p (b j) -> p b j", j=50)[:, None, :, :] \
                    .broadcast_to([50, 7, gb, 50])
                in1 = gt[:].rearrange("p (o j) -> p o j", j=50)[:, :, None, :] \
                    .broadcast_to([50, 7, gb, 50])
                nc.gpsimd.tensor_tensor(tmp[:], in0, in1, _mb.AluOpType.mult)
                red = rp.tile([50, 7 * gb], F32, tag="red")
                nc.vector.tensor_reduce(
                    red[:], tmp[:], axis=_mb.AxisListType.X, op=_mb.AluOpType.add,
                )
                nc.tensor.matmul(out_ps[:, out_off: out_off + 7 * gb],
                                 lhsT=on32[:], rhs=red[:], start=True, stop=True)

            # software pipeline: A(g+1) emitted before B(g); x prefetch depth 2
            n = len(CHUNKS)
            offs = np.cumsum([0] + [7 * gb for _, gb in CHUNKS]).tolist()
            xts_q = [emit_xdma(0), emit_xdma(1)]
            h_q = {}
            h_q[0] = do_groupA(0, xts_q[0])
            for gi in range(n):
                if gi + 2 < n:
                    xts_q.append(emit_xdma(gi + 2))
                if gi + 1 < n:
                    h_q[gi + 1] = do_groupA(gi + 1, xts_q[gi + 1])
                do_groupB(gi, h_q.pop(gi), offs[gi])

            o_sb = op_pool.tile([1, 7 * BC], F32, tag="osb")
            nc.scalar.copy(o_sb[:], out_ps[:])
            nc.sync.dma_start(out=o_d[:].rearrange("(a f) -> a f", a=1), in_=o_sb[:])

    _split_excess_waits(nc)
    return nc


def _get_program():
    if "nc" not in _CACHE:
        _apply_tile_patch()
        _CACHE["nc"] = _build_program()
    return _CACHE["nc"]


def _host_prep(W1, W2, W3, Wl, bl):
    W = (W1.astype(np.float64) @ W2.astype(np.float64) @ W3.astype(np.float64))
    Wstack = np.empty((100, 200), np.float16)
    for r in range(4):
        Wstack[:, 50 * r: 50 * r + 50] = W[100 * r: 100 * r + 100, :]

    iu, ju = np.triu_indices(N_OUT)
    G = np.zeros((7, N_OUT, N_OUT), np.float64)
    Wl64 = Wl.astype(np.float64)
    half = np.sqrt(2.0) / 2.0
    for k, (i, j) in enumerate(zip(iu, ju)):
        if i == j:
            G[:, i, j] = Wl64[:, k]
        else:
            G[:, i, j] = Wl64[:, k] * half
            G[:, j, i] = Wl64[:, k] * half
    # g tile [50, 350]: block o = G_o  (broadcast over the batch dim on device)
    gtile = np.empty((50, 350), np.float16)
    for o in range(7):
        gtile[:, 50 * o: 50 * o + 50] = G[o].astype(np.float16)

    a = np.array(COEF, np.float64)
    eye = np.eye(50, dtype=np.float32)
    ci = np.zeros((50, 450), np.float32)
    for k, cv in enumerate([-M_SHIFT, a[7], a[8], a[6], a[4], a[5], a[3], a[1], a[2]]):
        ci[:, 50 * k: 50 * k + 50] = np.float32(cv) * eye
    it8 = np.tile(eye, (1, 8)).astype(np.float16)

    bias = (bl.astype(np.float64) + a[0] * np.einsum("oii->o", G)).astype(np.float32)
    return Wstack, gtile, ci, it8, bias


def kernel(x, W1, W2, W3, Wl, bl):
    from concourse.bass_utils import run_bass_kernel_spmd

    x = np.asarray(x)
    W1, W2, W3 = np.asarray(W1), np.asarray(W2), np.asarray(W3)
    Wl, bl = np.asarray(Wl), np.asarray(bl)
    Wstack, gtile, ci, it8, bias = _host_prep(W1, W2, W3, Wl, bl)
    nc = _get_program()
    x = np.ascontiguousarray(x, np.float32)
    ones_col = np.ones((50, 1), np.float32)
    in_maps = [
        {"x": x[c * BC: (c + 1) * BC], "w": Wstack, "g": gtile, "ci": ci,
         "it": it8, "c32": ones_col}
        for c in range(N_CORES)
    ]
    res = run_bass_kernel_spmd(nc, in_maps, list(range(N_CORES)))
    outs = []
    for c in range(N_CORES):
        flat = res.results[c]["out"]  # chunked (o, bi) blocks per CHUNKS
        per_core = np.empty((BC, 7), np.float32)
        off = 0
        for (b0, gb) in CHUNKS:
            blk = flat[off: off + 7 * gb].reshape(7, gb)
            per_core[b0: b0 + gb] = blk.T
            off += 7 * gb
        outs.append(per_core)
    out = np.concatenate(outs, axis=0) + bias[None, :]
    return out.astype(np.float32)


if __name__ == "__main__":
    print("smoke build only")
